# revision 40
# baseline (speedup 1.0000x reference)
"""Trainium2 Bass kernel for a dense transformer block (pre-LN, 8-head causal
attention + FFN), data-parallel over batch across 8 NeuronCores.

Reference computation (B=128, T=256, C=384, H=8, HS=48):
    h  = LN(x; g1, beta1)
    q,k,v = per-head projections of h
    attn  = causal-softmax(q k^T / sqrt(HS)) v      (concat heads)
    x1 = x + attn @ Wproj + bproj
    h2 = LN(x1; g2, beta2)
    out = x1 + relu(h2 @ W1 + b1) @ W2 + b2

Sharding: batch 128 -> 16 sequences per core; all parameters replicated.

Design notes (v2 — PE-transpose + software-pipelined emission):
  * All matmul operands fp16, fp32 PSUM accumulation; LN affine folded into
    weights on the host (rank-1 beta terms emitted only when nonzero).
  * [t,c] <-> [c,t] layout changes via PE identity-matmul transposes
    (12 x [128,128] f16 per tensor) + one psum->sbuf copy per c-chunk.
    No DRAM scratch, no sync-engine transpose DMAs.
  * V stored augmented [t, 8*(48+1)]: softmax denominators fall out of the
    attention numerator matmuls (col 48 of each head block); the ones column
    is memset (or a rank-1 row matmul when beta1 != 0).
  * Causal mask applied as an additive -6e4 bias pre-loaded into the scores
    PSUM by an identity matmul, so the per-head chain is scores -> exp only.
  * Attention normalize is batched: one DVE reciprocal over the 8 den
    columns + one stride-0-broadcast multiply per token chunk (not per head).
  * LN: DVE bn_stats (LN2 stats ride the residual adds) + one-step
    bit-magic Newton rsqrt; LN1(g+1) is emitted mid-attention(g) where the
    DVE is idle.  (GpSimd is avoided entirely: its ALU ops measured 5-20x
    slower than DVE and poisoned the critical path.)
  * FFN(g) is emitted as filler units inside attention(g+1): the FFN matmuls
    plug the exp-wait gaps so the PE never idles and stays at its top
    p-state (idle gaps drop the PE clock 2.4 -> 1.2/0.65 GHz).  Relu runs
    on DVE (tensor_scalar_max) to keep Scalar's exp cadence undisturbed.
  * Emission order per group: attn(g) [+ FFN(g-1) filler + LN1(g+1) midway]
    -> attnT-transpose+proj(g)+LN2-stats -> LN2(g) -> h-transpose+QKV(g+1);
    engines execute in order, so emission order IS the schedule.
  * PSUM: 8 banks = sp(3: scores/proj/ffn2) + big(3: qkv/ffn1/transposes,
    mixing f32 [128,512] and f16 [128,1024] allocs in one tag)
    + nu(2: attention numerators, all 8 heads packed per token chunk).
"""

from collections import deque

import numpy as np

import concourse.bass as bass
import concourse.mybir as mybir
import concourse.tile as tile
from concourse import bacc
from concourse.bass_utils import run_bass_kernel_spmd

F32 = mybir.dt.float32
F16 = mybir.dt.float16
I32 = mybir.dt.int32

# Model dims
B, T, C = 128, 256, 384
H, HS = 8, 48
FF = 4 * C           # 1536
EPS = 1e-5

# Sharding / tiling
NCORES = 8
NB = B // NCORES     # 16 sequences per core
TOK = NB * T         # 4096 tokens per core
P = 128
CCH = C // P         # 3 c-chunks
FCH = FF // P        # 12 ffn chunks
DPAD = 512           # q/k head-padded dim (4 tiles x 2 heads x 64)
QMT = DPAD // P      # 4
HW1 = HS + 1         # 49
VW = H * HW1         # 392 augmented v width
GT = 512             # tokens per group (2 sequences)
NG = TOK // GT       # 8 groups
GTT = GT // P        # 4 token tiles per group
ISCALE = float(HS) ** -0.5
MAGIC = 0x5F3759DF
SKEW = 2             # head-level software pipeline depth in attention


def _build_program(flags):
    nc = bacc.Bacc(None, target_bir_lowering=False, debug=False)

    x_d = nc.dram_tensor("x", [TOK, C], F32, kind="ExternalInput").ap()
    wq_d = nc.dram_tensor("wq", [CCH, P, DPAD], F16, kind="ExternalInput").ap()
    wk_d = nc.dram_tensor("wk", [CCH, P, DPAD], F16, kind="ExternalInput").ap()
    wv_d = nc.dram_tensor("wv", [CCH, P, VW], F16, kind="ExternalInput").ap()
    wp_d = nc.dram_tensor("wp", [CCH, P, C], F16, kind="ExternalInput").ap()
    w1_d = nc.dram_tensor("w1", [CCH, P, FF], F16, kind="ExternalInput").ap()
    w2_d = nc.dram_tensor("w2", [FCH, P, C], F16, kind="ExternalInput").ap()
    rowq_d = nc.dram_tensor("rowq", [1, DPAD], F16, kind="ExternalInput").ap()
    rowk_d = nc.dram_tensor("rowk", [1, DPAD], F16, kind="ExternalInput").ap()
    rowv_d = nc.dram_tensor("rowv", [1, VW], F16, kind="ExternalInput").ap()
    rowp_d = nc.dram_tensor("rowp", [1, C], F16, kind="ExternalInput").ap()
    rowl_d = nc.dram_tensor("rowl", [1, C], F16, kind="ExternalInput").ap()
    b1t_d = nc.dram_tensor("b1t", [P, FCH], F32, kind="ExternalInput").ap()
    mask_d = nc.dram_tensor("maskmul", [P, 3 * P], F16, kind="ExternalInput").ap()
    ident_d = nc.dram_tensor("ident", [P, P], F16, kind="ExternalInput").ap()
    out_d = nc.dram_tensor("out", [TOK, C], F32, kind="ExternalOutput").ap()

    with tile.TileContext(nc) as tc:
        _emit(nc, tc, flags, x_d, wq_d, wk_d, wv_d, wp_d, w1_d, w2_d,
              rowq_d, rowk_d, rowv_d, rowp_d, rowl_d, b1t_d, mask_d, ident_d,
              out_d)
    nc.compile()
    return nc


def _emit(nc, tc, flags, x_d, wq_d, wk_d, wv_d, wp_d, w1_d, w2_d,
          rowq_d, rowk_d, rowv_d, rowp_d, rowl_d, b1t_d, mask_d, ident_d,
          out_d):
    from contextlib import ExitStack
    with ExitStack() as ctx:
        const = ctx.enter_context(tc.tile_pool(name="const", bufs=1))
        ln = ctx.enter_context(tc.tile_pool(name="ln", bufs=8))
        grp = ctx.enter_context(tc.tile_pool(name="grp", bufs=2))
        att = ctx.enter_context(tc.tile_pool(name="att", bufs=8))
        outp = ctx.enter_context(tc.tile_pool(name="outp", bufs=4))
        psum = ctx.enter_context(tc.tile_pool(name="psum", bufs=3, space="PSUM"))

        # ---- constants ----
        wq_sb = const.tile([P, CCH, DPAD], F16)
        wk_sb = const.tile([P, CCH, DPAD], F16)
        wv_sb = const.tile([P, CCH, VW], F16)
        wp_sb = const.tile([P, CCH, C], F16)
        w1_sb = const.tile([P, CCH, FF], F16)
        w2_sb = const.tile([P, FCH, C], F16)
        mask_sb = const.tile([P, 3 * P], F16)
        ident_sb = const.tile([P, P], F16)
        weight_dmas = [lambda: nc.sync.dma_start(ident_sb, ident_d),
                       lambda: nc.sync.dma_start(mask_sb, mask_d)]
        for cc in range(CCH):
            weight_dmas.append(
                lambda cc=cc: nc.sync.dma_start(wq_sb[:, cc, :], wq_d[cc]))
            weight_dmas.append(
                lambda cc=cc: nc.sync.dma_start(wk_sb[:, cc, :], wk_d[cc]))
            weight_dmas.append(
                lambda cc=cc: nc.sync.dma_start(wv_sb[:, cc, :], wv_d[cc]))
        late_dmas = []
        for cc in range(CCH):
            late_dmas.append(
                lambda cc=cc: nc.sync.dma_start(wp_sb[:, cc, :], wp_d[cc]))
            late_dmas.append(
                lambda cc=cc: nc.sync.dma_start(w1_sb[:, cc, :], w1_d[cc]))
        for fc in range(FCH):
            late_dmas.append(
                lambda fc=fc: nc.sync.dma_start(w2_sb[:, fc, :], w2_d[fc]))

        ones_sb = const.tile([1, GT], F16)
        nc.vector.memset(ones_sb, 1.0)
        rowq_sb = const.tile([1, DPAD], F16)
        rowk_sb = const.tile([1, DPAD], F16)
        rowv_sb = const.tile([1, VW], F16)
        rowp_sb = const.tile([1, C], F16)
        rowl_sb = const.tile([1, C], F16)
        b1t_sb = const.tile([P, FCH], F32)
        if flags["rowq"]:
            nc.sync.dma_start(rowq_sb, rowq_d)
        if flags["rowk"]:
            nc.sync.dma_start(rowk_sb, rowk_d)
        if flags["rowv"]:
            nc.sync.dma_start(rowv_sb, rowv_d)
        if flags["rowp"]:
            nc.sync.dma_start(rowp_sb, rowp_d)
        if flags["rowl"]:
            nc.sync.dma_start(rowl_sb, rowl_d)
        if flags["b1t"]:
            nc.sync.dma_start(b1t_sb, b1t_d)

        def ln_stats(src, mv4, i):
            stats = ln.tile([P, 6], F32, tag="stats")
            nc.vector.bn_stats(out=stats, in_=src)
            nc.vector.bn_aggr(out=mv4[:, i, :], in_=stats)

        def ln_finish(mv4, tiles, dsts):
            """Newton-rsqrt from collected stats + normalize (all DVE)."""
            # rstd = rsqrt(var + eps): bit-magic init + 2 Newton steps (DVE)
            ve = ln.tile([P, GTT], F32, tag="ve")
            y = ln.tile([P, GTT], F32, tag="y")
            t = ln.tile([P, GTT], F32, tag="t")
            nc.vector.tensor_scalar_add(ve, mv4[:, :, 1], EPS)
            vi = ve.bitcast(I32)
            yi = y.bitcast(I32)
            nc.vector.tensor_scalar(out=yi, in0=vi, scalar1=1, scalar2=0,
                                    op0=mybir.AluOpType.arith_shift_right,
                                    op1=mybir.AluOpType.arith_shift_right)
            nc.vector.tensor_scalar(out=yi, in0=yi, scalar1=-1, scalar2=MAGIC,
                                    op0=mybir.AluOpType.mult,
                                    op1=mybir.AluOpType.add)
            for _ in range(1):
                nc.vector.tensor_mul(t, y, y)
                nc.vector.tensor_mul(t, t, ve)
                nc.vector.tensor_scalar(out=t, in0=t, scalar1=-0.5, scalar2=1.5,
                                        op0=mybir.AluOpType.mult,
                                        op1=mybir.AluOpType.add)
                nc.vector.tensor_mul(y, y, t)
            for i, (src, dst) in enumerate(zip(tiles, dsts)):
                nc.vector.tensor_scalar(out=dst, in0=src,
                                        scalar1=mv4[:, i, 0:1],
                                        scalar2=y[:, i:i + 1],
                                        op0=mybir.AluOpType.subtract,
                                        op1=mybir.AluOpType.mult)

        def ln_group(tiles, dsts):
            mv4 = ln.tile([P, GTT, 2], F32, tag="mv4")
            for i, src in enumerate(tiles):
                ln_stats(src, mv4, i)
            ln_finish(mv4, tiles, dsts)

        def trans3(dst, srcs, engs):
            """dst[:, cc, :GT] (f16 sbuf) <- transpose of 4 [P, C] f16 tiles.

            engs: iterable of 's'/'v' choosing the psum->sbuf copy engine.
            """
            for ccs in ((0, 1), (2,)):
                tp = psum.tile([P, 2 * GT], F16, tag="big", name="tp", bufs=4)
                for k, cc in enumerate(ccs):
                    for tt in range(GTT):
                        nc.tensor.transpose(
                            tp[:, k * GT + tt * P:k * GT + (tt + 1) * P],
                            srcs[tt][:, cc * P:(cc + 1) * P],
                            ident_sb)
                    # copy each cc as soon as its transposes are emitted, so
                    # the first consumer matmul isn't left waiting
                    if next(engs) == 's':
                        nc.scalar.copy(dst[:, cc, :], tp[:, k * GT:(k + 1) * GT])
                    else:
                        nc.vector.tensor_copy(dst[:, cc, :],
                                              tp[:, k * GT:(k + 1) * GT])

        state = {}

        def stage_load(g):
            if g >= NG:
                return
            xg = grp.tile([P, GTT, C], F32, tag="xg", name="xg", bufs=3)
            for tt in range(GTT):
                it = g * GTT + tt
                nc.sync.dma_start(xg[:, tt, :], x_d[it * P:(it + 1) * P, :])
            state[g] = {"xg": xg}

        def stage_ln1(g):
            if g >= NG:
                return
            st = state[g]
            hNs = [ln.tile([P, C], F16, tag="hN", name="hN")
                   for _ in range(GTT)]
            ln_group([st["xg"][:, tt, :] for tt in range(GTT)], hNs)
            st["hN"] = hNs

        def stage_qkv(g):
            if g >= NG:
                return
            st = state[g]
            hT = grp.tile([P, CCH, GT], F16, tag="hT", name="hT")
            trans3(hT, st["hN"], iter(('s', 'v', 's')))
            qT = grp.tile([P, QMT, GT], F16, tag="qT", name="qT")
            kT = grp.tile([P, QMT, GT], F16, tag="kT", name="kT")
            vaug = grp.tile([P, GTT, VW], F16, tag="vaug", name="vaug")
            for dst, w_sb, row_sb, rowf, ceng in (
                    (qT, wq_sb, rowq_sb, flags["rowq"], 's'),
                    (kT, wk_sb, rowk_sb, flags["rowk"], 'v')):
                for m in range(QMT):
                    ps = psum.tile([P, GT], F32, tag="big", name="ps", bufs=4)
                    for cc in range(CCH):
                        nc.tensor.matmul(ps, lhsT=w_sb[:, cc, m * P:(m + 1) * P],
                                         rhs=hT[:, cc, :],
                                         start=(cc == 0),
                                         stop=(cc == CCH - 1 and not rowf))
                    if rowf:
                        nc.tensor.matmul(ps, lhsT=row_sb[:, m * P:(m + 1) * P],
                                         rhs=ones_sb, start=False, stop=True)
                    if ceng == 's':
                        nc.scalar.copy(dst[:, m, :], ps)
                    else:
                        nc.vector.tensor_copy(dst[:, m, :], ps)
            # V augmented; ones column via rank-1 row matmul when beta1 != 0,
            # else memset directly (saves a K=1 matmul per tile)
            for st_i in range(GTT):
                ps = psum.tile([P, GT], F32, tag="big", name="ps", bufs=4)
                for cc in range(CCH):
                    nc.tensor.matmul(ps[:, :VW],
                                     lhsT=hT[:, cc, st_i * P:(st_i + 1) * P],
                                     rhs=wv_sb[:, cc, :],
                                     start=(cc == 0),
                                     stop=(cc == CCH - 1 and not flags["rowv"]))
                if flags["rowv"]:
                    nc.tensor.matmul(ps[:, :VW], lhsT=ones_sb[:, :P],
                                     rhs=rowv_sb, start=False, stop=True)
                    nc.vector.tensor_copy(vaug[:, st_i, :], ps[:, :VW])
                else:
                    v3 = vaug[:, st_i, :].rearrange("p (h w) -> p h w", w=HW1)
                    nc.vector.tensor_copy(
                        v3[:, :, :HS],
                        ps[:, :VW].rearrange("p (h w) -> p h w", w=HW1)[:, :, :HS])
                    nc.vector.memset(v3[:, :, HS], 1.0)
            st.update(qT=qT, kT=kT, vaug=vaug)

        def stage_attn(g, midway=None, filler=None):
            st = state[g]
            qT, kT, vaug = st["qT"], st["kT"], st["vaug"]
            attn_ns = []
            for b2 in range(2):
                s0 = b2 * T
                nu_t = [psum.tile([P, VW], F32, tag="nu", name="nu", bufs=2)
                        for _ in range(2)]
                ews = [None] * H

                def sc(h):
                    m, hh = divmod(h, 2)
                    off = 64 * hh
                    sp = psum.tile([P, 3 * P], F32, tag="sp", name="sp", bufs=2)
                    # causal mask as additive bias (-6e4 above the diagonal),
                    # loaded into PSUM by an identity matmul so the per-head
                    # chain is just scores -> exp
                    nc.tensor.matmul(sp, lhsT=ident_sb, rhs=mask_sb,
                                     start=True, stop=False)
                    nc.tensor.matmul(
                        sp[:, :T],
                        lhsT=kT[off:off + HS, m, s0:s0 + P],
                        rhs=qT[off:off + HS, m, s0:s0 + T],
                        start=False, stop=False)
                    nc.tensor.matmul(
                        sp[:, T:T + P],
                        lhsT=kT[off:off + HS, m, s0 + P:s0 + T],
                        rhs=qT[off:off + HS, m, s0 + P:s0 + T],
                        start=False, stop=True)
                    ew = att.tile([P, 3 * P], F16, tag="ew", name="ew", bufs=10)
                    nc.scalar.activation(out=ew, in_=sp,
                                         func=mybir.ActivationFunctionType.Exp,
                                         scale=ISCALE)
                    ews[h] = ew

                def nu0(h):
                    hs = h * HW1
                    nc.tensor.matmul(nu_t[0][:, hs:hs + HW1],
                                     lhsT=ews[h][:, :P],
                                     rhs=vaug[:, b2 * 2, hs:hs + HW1],
                                     start=True, stop=True)

                def nu1(h):
                    hs = h * HW1
                    nc.tensor.matmul(nu_t[1][:, hs:hs + HW1],
                                     lhsT=ews[h][:, P:T],
                                     rhs=vaug[:, b2 * 2, hs:hs + HW1],
                                     start=True, stop=False)
                    nc.tensor.matmul(nu_t[1][:, hs:hs + HW1],
                                     lhsT=ews[h][:, T:T + P],
                                     rhs=vaug[:, b2 * 2 + 1, hs:hs + HW1],
                                     start=False, stop=True)

                def recip_norm(tch):
                    nu3 = nu_t[tch].rearrange("p (h w) -> p h w", w=HW1)
                    rec = att.tile([P, H], F32, tag="rec", name="rec")
                    rec3 = rec.rearrange("p (h o) -> p h o", o=1)
                    nc.vector.reciprocal(out=rec3, in_=nu3[:, :, HS:HS + 1])
                    an = att.tile([P, C], F16, tag="an", name="an")
                    an3 = an.rearrange("p (h w) -> p h w", w=HS)
                    b0, b1 = bass.broadcast_tensor_aps(nu3[:, :, 0:HS], rec3)
                    nc.vector.tensor_mul(an3, b0, b1)
                    attn_ns.append(an)

                # pipeline: scores/exp skewed against tchunk-0 numerators,
                # then tchunk-1 numerators cover the tchunk-0 normalize;
                # FFN units of the previous group fill the exp-wait gaps
                for step in range(H + SKEW):
                    if step < H:
                        sc(step)
                    if filler and step % 2 == 0:
                        filler.popleft()()
                    if step >= SKEW:
                        nu0(step - SKEW)
                recip_norm(0)
                for h in range(H):
                    nu1(h)
                    if filler and h % 2 == 0:
                        filler.popleft()()
                recip_norm(1)
                if b2 == 0 and midway is not None:
                    midway()   # LN1(g+1) on the now-idle DVE mid-attention
            st["an"] = attn_ns

        def stage_proj(g):
            st = state[g]
            attnT = grp.tile([P, CCH, GT], F16, tag="attnT", name="attnT")
            trans3(attnT, st["an"], iter(('v', 's', 'v')))
            xg = st["xg"]
            mv4 = ln.tile([P, GTT, 2], F32, tag="mv4")
            st["mv4"] = mv4
            for tt in range(GTT):
                ps = psum.tile([P, 3 * P], F32, tag="sp", name="ps", bufs=2)
                for cc in range(CCH):
                    nc.tensor.matmul(ps[:, :C],
                                     lhsT=attnT[:, cc, tt * P:(tt + 1) * P],
                                     rhs=wp_sb[:, cc, :],
                                     start=(cc == 0),
                                     stop=(cc == CCH - 1 and not flags["rowp"]))
                if flags["rowp"]:
                    nc.tensor.matmul(ps[:, :C], lhsT=ones_sb[:, :P], rhs=rowp_sb,
                                     start=False, stop=True)
                nc.vector.tensor_add(xg[:, tt, :], xg[:, tt, :], ps[:, :C])
                ln_stats(xg[:, tt, :], mv4, tt)   # LN2 stats ride the residual

        def stage_ln2(g):
            st = state[g]
            h2s = [ln.tile([P, C], F16, tag="h2", name="h2")
                   for _ in range(GTT)]
            ln_finish(st["mv4"], [st["xg"][:, tt, :] for tt in range(GTT)], h2s)
            st["h2"] = h2s

        def make_ffn_units(g):
            """FFN for group g as a deque of emission closures, consumed as
            PE filler inside the next group's attention."""
            st = state[g]
            h2T = grp.tile([P, CCH, GT], F16, tag="h2T", name="h2T")
            rg = grp.tile([P, FCH, GT], F16, tag="rg", name="rg")
            units = deque()

            def u_h2tr():
                trans3(h2T, st["h2"], iter(('s', 'v', 's')))
            units.append(u_h2tr)

            def u_ffn1(fc):
                ps = psum.tile([P, GT], F32, tag="big", name="ps", bufs=4)
                for cc in range(CCH):
                    nc.tensor.matmul(ps, lhsT=w1_sb[:, cc, fc * P:(fc + 1) * P],
                                     rhs=h2T[:, cc, :],
                                     start=(cc == 0), stop=(cc == CCH - 1))
                if flags["b1t"]:
                    nc.scalar.activation(out=rg[:, fc, :], in_=ps,
                                         func=mybir.ActivationFunctionType.Relu,
                                         bias=b1t_sb[:, fc:fc + 1], scale=1.0)
                else:
                    nc.vector.tensor_scalar_max(rg[:, fc, :], ps, 0.0)
            for fc in range(FCH):
                units.append(lambda fc=fc: u_ffn1(fc))

            def u_ffn2(tt):
                it = g * GTT + tt
                ps = psum.tile([P, 3 * P], F32, tag="sp", name="ps", bufs=2)
                for fc in range(FCH):
                    nc.tensor.matmul(ps[:, :C],
                                     lhsT=rg[:, fc, tt * P:(tt + 1) * P],
                                     rhs=w2_sb[:, fc, :],
                                     start=(fc == 0),
                                     stop=(fc == FCH - 1 and not flags["rowl"]))
                if flags["rowl"]:
                    nc.tensor.matmul(ps[:, :C], lhsT=ones_sb[:, :P], rhs=rowl_sb,
                                     start=False, stop=True)
                ot = outp.tile([P, C], F32, tag="ot", name="ot")
                nc.vector.tensor_add(ot, st["xg"][:, tt, :], ps[:, :C])
                nc.sync.dma_start(out_d[it * P:(it + 1) * P, :], ot)
            for tt in range(GTT):
                units.append(lambda tt=tt: u_ffn2(tt))
            return units

        # ================= software-pipelined schedule =================
        stage_load(0)
        for d in weight_dmas:
            d()
        stage_load(1)
        stage_ln1(0)
        for d in late_dmas:
            d()
        stage_qkv(0)
        units = deque()
        for g in range(NG):
            stage_load(g + 2)
            stage_attn(g, midway=lambda: stage_ln1(g + 1), filler=units)
            while units:
                units.popleft()()
            stage_proj(g)
            stage_ln2(g)
            stage_qkv(g + 1)
            units = make_ffn_units(g)
        while units:
            units.popleft()()


def _prep_weights(Wq, Wk, Wv, Wproj, bproj, W1, b1, W2, b2, g1, beta1, g2, beta2):
    f16 = np.float16
    g1 = g1.astype(np.float64)
    g2 = g2.astype(np.float64)

    def qk_pack(W):
        Ws = g1[None, :, None] * W.astype(np.float64)      # [H, C, HS]
        pad = np.zeros((CCH, P, DPAD), np.float64)
        row = np.zeros((1, DPAD), np.float64)
        # beta1 @ W uses the unscaled W: h_aff@W = h_norm@(g1*W) + beta1@W
        beta_r = np.einsum('c,hcd->hd', beta1.astype(np.float64),
                           W.astype(np.float64))
        for h in range(H):
            m, hh = divmod(h, 2)
            col = m * P + 64 * hh
            pad[:, :, col:col + HS] = Ws[h].reshape(CCH, P, HS)
            row[0, col:col + HS] = beta_r[h]
        return pad.astype(f16), row.astype(f16)

    wq_pad, rowq = qk_pack(Wq)
    wk_pad, rowk = qk_pack(Wk)

    # V: augmented per-head layout [c, h*(HS+1)]; ones col via rank-1 row
    Wvs = (g1[None, :, None] * Wv.astype(np.float64))       # [H, C, HS]
    beta_v = np.einsum('c,hcd->hd', beta1.astype(np.float64),
                       Wv.astype(np.float64))
    wv = np.zeros((C, VW), np.float64)
    rowv = np.zeros((1, VW), np.float64)
    for h in range(H):
        wv[:, h * HW1:h * HW1 + HS] = Wvs[h]
        rowv[0, h * HW1:h * HW1 + HS] = beta_v[h]
        rowv[0, h * HW1 + HS] = 1.0
    wv = wv.astype(f16).reshape(CCH, P, VW)
    rowv_flag = bool(np.any(beta_v != 0))
    rowv = rowv.astype(f16)

    wp = Wproj.astype(f16).reshape(CCH, P, C)
    rowp = bproj.astype(f16).reshape(1, C)

    W1s = g2[:, None] * W1.astype(np.float64)
    w1p = W1s.astype(f16).reshape(CCH, P, FF)
    b1tot = (b1.astype(np.float64)
             + beta2.astype(np.float64) @ W1.astype(np.float64))
    b1t = b1tot.astype(np.float32).reshape(FCH, P).T.copy()   # [P, FCH]

    w2p = W2.astype(f16).reshape(FCH, P, C)
    rowl = b2.astype(f16).reshape(1, C)

    # additive causal mask bias [s, t]: 0 where s <= t, -6e4 above
    tri = np.triu(np.ones((P, P), np.float64))
    trib = np.where(tri > 0, 0.0, -60000.0)
    maskmul = np.concatenate([trib, np.zeros((P, P)), trib], axis=1).astype(f16)
    ident = np.eye(P, dtype=f16)
    wdict = dict(wq=wq_pad, wk=wk_pad, wv=wv, wp=wp, w1=w1p, w2=w2p,
                 rowq=rowq, rowk=rowk, rowv=rowv, rowp=rowp, rowl=rowl,
                 b1t=b1t, maskmul=maskmul, ident=ident)
    flags = {k: bool(np.any(wdict[k] != 0))
             for k in ("rowq", "rowk", "rowp", "rowl", "b1t")}
    flags["rowv"] = rowv_flag
    return wdict, flags


_CACHED = {}


def _get_program(flags):
    key = tuple(sorted(flags.items()))
    if key not in _CACHED:
        _CACHED[key] = _build_program(flags)
    return _CACHED[key]


def _run(inputs, trace=False):
    x = np.asarray(inputs["x"], np.float32)
    wdict, flags = _prep_weights(
        np.asarray(inputs["Wq"]), np.asarray(inputs["Wk"]),
        np.asarray(inputs["Wv"]), np.asarray(inputs["Wproj"]),
        np.asarray(inputs["bproj"]), np.asarray(inputs["W1"]),
        np.asarray(inputs["b1"]), np.asarray(inputs["W2"]),
        np.asarray(inputs["b2"]), np.asarray(inputs["g1"]),
        np.asarray(inputs["beta1"]), np.asarray(inputs["g2"]),
        np.asarray(inputs["beta2"]))

    shards = x.reshape(NCORES, NB * T, C)
    in_maps = [dict(wdict, x=np.ascontiguousarray(shards[i]))
               for i in range(NCORES)]
    nc = _get_program(flags)
    res = run_bass_kernel_spmd(nc, in_maps, list(range(NCORES)), trace=trace)
    out = np.stack([res.results[i]["out"] for i in range(NCORES)])
    return out.reshape(B, T, C).astype(np.float32), res


def kernel(**inputs):
    out, _ = _run(inputs, trace=False)
    return out
